# revision 1
# baseline (speedup 1.0000x reference)
# Contrastive loss (L2-distance scores, margin hinge, mean reduction) on 8
# Trainium2 NeuronCores.
#
# total = mean(cost_s) + mean(cost_im) over the [N, N] score matrix
#   D_ij = ||im_i - s_j||;  a_i = b_i = margin + D_ii (host-exact)
#   sum(cost_s) + sum(cost_im) = sum_ij [relu(a_i - D_ij) + relu(b_j - D_ij)]
#   (diagonal contributes ~2*margin each; subtracted exactly on host)
#
# PE: one fp8e4 DoubleRow matmul (K=256) per 512-col chunk computes
# (-im8).s8; a second K=8 fp8 DoubleRow "fold" matmul adds the TRUE norms
# imsq_i/2 + ssq_j/2 (4-term fp8 residual splits; using exact norms makes the
# quantization of the cross term unbiased), so PSUM q = d2/2 estimate.
#
# The 32 [128 x 2048] tiles per core are split over three elementwise
# pipelines so ACT, DVE and Pool (gpsimd) all carry the work:
#   B: ACT sqrt(2q) -> D~ bf16; DVE custom hinge
#        out = max(D,a) + max(D,b) - 2D, accum row-sum
#   C: ACT sqrt(2q) + accum(R3=sum D); Pool STT R1 = sum max(D,a);
#        Pool STT R2 = sum max(D,b); host adds R1+R2-2*R3
#   N: two custom DVE ops straight from PSUM:
#        P = m*h1, m = min(q, clamp), h1 = Newton step from linear rsqrt seed
#        => P ~ sqrt(min(d2, clamp^2... )/4) = min(clamp_D, D)/2, accum.
#        clamp = a_i^2/2 (per-partition scalar) or b_j^2/2 (bf16 row tile).
#        host: term = sum a_i * Ncols + sum b_j * Nrows - 2(1+c)*sum P
#        (c = hardcoded Newton undershoot centering, ~2.6e-3)
#
# Final: per-core accumulator block [128, NACC] fp32 DMA'd out; host combines
# in float64 with per-column weights, subtracts 2*margin*N, divides by N^2.

import os

import numpy as np
import ml_dtypes

import concourse.bass as bass
import concourse.tile as tile
from concourse import bacc, mybir
from concourse import bass_utils
from concourse import dve_ops as _dve_ops
from concourse.dve_spec import (
    Spec as _DveSpec,
    Src0,
    Src1,
    C0,
    C1,
    C2 as _C2,
    C3 as _C3,
    lower as _dve_lower,
    maxx as _dve_maxx,
    minn as _dve_minn,
    _spill_c3_to_src1,
)
from concourse.dve_uop import DveOpSpec as _DveOpSpec

N = 8192
D = 256
MARGIN = 0.2
NCORES = 8
SLAB = N // NCORES          # 1024 rows per core
MT = SLAB // 128            # 8 m-tiles per core
GROUP = 2048                # columns per PSUM group (4 banks)
NG = N // GROUP             # 4 groups
CHUNK = 512                 # columns per matmul (1 PSUM bank)
NCHUNK = GROUP // CHUNK     # 4 chunks per group

# rsqrt seed (minimax linear fit of (2m)^-1/2 on m in [136, 360]) + 1 Newton
SEED_I = 697.09065447877
SEED_B = 0.00010431297604503012
# Newton undershoot centering (host-side multiplicative correction on sum P)
CA = 0.00259083762787915
CB = 0.0025867461933839486

# pipeline per (g, m) tile: 4 groups x 8 m-tiles, emitted g-major.
# B=11 (ACT+DVE), C=15 (ACT+2xPool), N=6 (2xDVE from PSUM)
# NOTE: walrus rejects TensorScalarPtr on the Pool engine (V3 ISA), so the
# "C" pipeline is unavailable on hardware; default everything through "B"
# with the term-B hinge on DVE. "N" tiles only relieve ACT (they add DVE
# load), so they are off by default too.
PIPE = os.environ.get("PIPE", "B" * 32)
assert len(PIPE) == NG * MT

BF16 = ml_dtypes.bfloat16
_F = mybir.dt.float32
_B = mybir.dt.bfloat16
_P8 = mybir.dt.float8e4


def _register_ops():
    """Register the three custom DVE ops (idempotent)."""
    names = {op.name: op for op in _dve_ops.OPS}
    out = []

    def _add_op(name, spec):
        if name in names:
            out.append(names[name])
            return
        shas = {}
        for ver in ("v3", "v4"):
            try:
                s = _DveOpSpec(
                    name=name, opcode=0, uops=_dve_lower(spec, ver=ver), rd1_en=True
                )
                shas[ver] = s.sha(ver)
            except Exception:
                pass
        op = _dve_ops.DveOp(name, spec, subdim=False, uops_sha=shas)
        _dve_ops.OPS.append(op)
        _dve_ops._SUB_OPCODE_FOR_NAME[op.name] = (
            _dve_ops._CUSTOM_DVE_ROW_BASE + len(_dve_ops.OPS) - 1
        )
        out.append(op)

    from operator import add as _addf

    # --- hinge: out = max(Src0, C0) + max(Src0, Src1) - Src0*C2, accum add
    def _ref_h(in0, in1, s0, s1, imm2):
        x = in0.astype(np.float32)
        body = (
            np.maximum(x, s0)
            + np.maximum(x, in1.astype(np.float32))
            - x * imm2
        ).astype(np.float32)
        return body, body.reshape(body.shape[0], -1).sum(axis=-1, keepdims=True)

    _add_op(
        "CL2_HINGE_ANT",
        _DveSpec(
            body=_dve_maxx(Src0, C0) + _dve_maxx(Src0, Src1) - Src0 * _C2,
            accum=_addf,
            reference=_ref_h,
        ),
    )

    # --- sqmin A: m = min(Src0, C0); seed h0 = C1*(C3 - m) [C3 via in1];
    #     g0 = m*h0; u = g0*h0; v = C2 - u; P = g0*v ~= sqrt(m/2); accum add
    def _mk_body(m, icpt):
        e = icpt - m
        h0 = C1 * e
        g0 = m * h0
        u = g0 * h0
        v = _C2 - u
        return g0 * v

    def _ref_a(in0, in1, s0, s1, imm2):
        mm = np.minimum(in0.astype(np.float32), s0).astype(np.float32)
        ee = (in1.astype(np.float32) - mm).astype(np.float32)
        hh0 = (s1 * ee).astype(np.float32)
        gg = (mm * hh0).astype(np.float32)
        uu = (gg * hh0).astype(np.float32)
        vv = (imm2 - uu).astype(np.float32)
        body = (gg * vv).astype(np.float32)
        return body, body.reshape(body.shape[0], -1).sum(axis=-1, keepdims=True)

    _add_op(
        "CL2_SQMINA_ANT",
        _DveSpec(
            body=_spill_c3_to_src1(_mk_body(_dve_minn(Src0, C0), _C3)),
            accum=_addf,
            reference=_ref_a,
        ),
    )

    # --- sqmin B: m = min(Src0, Src1); seed h0 = C1*(C0 - m); ... accum add
    def _ref_b(in0, in1, s0, s1, imm2):
        mm = np.minimum(in0.astype(np.float32), in1.astype(np.float32)).astype(
            np.float32
        )
        ee = (s0 - mm).astype(np.float32)
        hh0 = (s1 * ee).astype(np.float32)
        gg = (mm * hh0).astype(np.float32)
        uu = (gg * hh0).astype(np.float32)
        vv = (imm2 - uu).astype(np.float32)
        body = (gg * vv).astype(np.float32)
        return body, body.reshape(body.shape[0], -1).sum(axis=-1, keepdims=True)

    _add_op(
        "CL2_SQMINB_ANT",
        _DveSpec(
            body=_mk_body(_dve_minn(Src0, Src1), C0),
            accum=_addf,
            reference=_ref_b,
        ),
    )
    return out


def _acc_layout():
    """(col base, ncols) per tile in emission order + total col count."""
    bases = []
    c = 0
    for ch in PIPE:
        n = {"B": 1, "C": 3, "N": 2}[ch]
        bases.append(c)
        c += n
    return bases, c


def build_module():
    nc = bacc.Bacc("TRN2", num_devices=NCORES)
    op_h, op_a, op_b = _register_ops()

    imT = nc.dram_tensor("imT", [2, 128, SLAB], _P8, kind="ExternalInput")
    sT = nc.dram_tensor("sT", [2, 128, N], _P8, kind="ExternalInput")
    foldr = nc.dram_tensor("foldr", [4, 2, N], _P8, kind="ExternalInput")
    foldl = nc.dram_tensor("foldl", [4, 2, SLAB], _P8, kind="ExternalInput")
    brow = nc.dram_tensor("brow", [N], _B, kind="ExternalInput")
    b2row = nc.dram_tensor("b2row", [N], _B, kind="ExternalInput")
    avec = nc.dram_tensor("avec", [128, MT], _F, kind="ExternalInput")
    a2vec = nc.dram_tensor("a2vec", [128, MT], _F, kind="ExternalInput")
    bases, nacc = _acc_layout()
    out = nc.dram_tensor("out", [128, nacc], _F, kind="ExternalOutput")

    dbufs = int(os.environ.get("DBUFS", "3"))
    tbufs = int(os.environ.get("TBUFS", "2"))
    AL = mybir.AluOpType

    with tile.TileContext(nc) as tc:
        with (
            tc.tile_pool(name="singles", bufs=1) as singles,
            tc.tile_pool(name="dtiles", bufs=dbufs) as dpool,
            tc.tile_pool(name="trash", bufs=tbufs) as tpool,
            tc.tile_pool(name="psum", bufs=2, space="PSUM") as ppool,
        ):
            # ---- input DMAs (all on the sync/HWDGE queue; Pool kept free) --
            lhs_sb = singles.tile([128, 2, SLAB], _P8)
            rhs_sb = singles.tile([128, 2, N], _P8)
            foldr_sb = singles.tile([128, 2, N], _P8)
            foldl_sb = singles.tile([128, 2, SLAB], _P8)
            b_sb = singles.tile([128, N], _B)
            b2_sb = singles.tile([128, N], _B)
            avec_sb = singles.tile([128, MT], _F)
            a2vec_sb = singles.tile([128, MT], _F)
            icol_sb = singles.tile([128, 1], _F)
            acc = singles.tile([128, nacc], _F)

            nc.sync.dma_start(out=lhs_sb[:, 0, :], in_=imT.ap()[0])
            nc.sync.dma_start(out=lhs_sb[:, 1, :], in_=imT.ap()[1])
            cols0 = slice(0, GROUP)
            nc.sync.dma_start(out=rhs_sb[:, 0, cols0], in_=sT.ap()[0, :, cols0])
            nc.sync.dma_start(out=rhs_sb[:, 1, cols0], in_=sT.ap()[1, :, cols0])
            for bp in (0, 32, 64, 96):
                nc.sync.dma_start(out=foldr_sb[bp : bp + 4, :, :], in_=foldr.ap())
                nc.sync.dma_start(out=foldl_sb[bp : bp + 4, :, :], in_=foldl.ap())
            nc.sync.dma_start(out=avec_sb[:], in_=avec.ap())
            if "N" in PIPE:
                nc.sync.dma_start(out=a2vec_sb[:], in_=a2vec.ap())
                nc.vector.memset(icol_sb[:], float(SEED_I))

            def bcast(dst, src_t, g):
                nc.sync.dma_start(
                    out=dst[:, g * GROUP : (g + 1) * GROUP],
                    in_=bass.AP(
                        tensor=src_t.ap().tensor,
                        offset=g * GROUP,
                        ap=[[0, 128], [1, GROUP]],
                    ),
                )

            bcast(b_sb, brow, 0)
            if "N" in PIPE:
                bcast(b2_sb, b2row, 0)
            for g in range(1, NG):
                cols = slice(g * GROUP, (g + 1) * GROUP)
                nc.sync.dma_start(out=rhs_sb[:, 0, cols], in_=sT.ap()[0, :, cols])
                nc.sync.dma_start(out=rhs_sb[:, 1, cols], in_=sT.ap()[1, :, cols])
                bcast(b_sb, brow, g)
                if "N" in PIPE:
                    bcast(b2_sb, b2row, g)

            # ---- main loop --------------------------------------------------
            for g in range(NG):
                for m in range(MT):
                    t = g * MT + m
                    pipe = PIPE[t]
                    cb = bases[t]
                    a_col = avec_sb[:, m : m + 1]
                    a2_col = a2vec_sb[:, m : m + 1]
                    bslice = b_sb[:, g * GROUP : (g + 1) * GROUP]
                    b2slice = b2_sb[:, g * GROUP : (g + 1) * GROUP]

                    ps = ppool.tile([128, GROUP], _F, tag="psum")
                    for c in range(NCHUNK):
                        pslice = ps[:, c * CHUNK : (c + 1) * CHUNK]
                        cols = slice(
                            g * GROUP + c * CHUNK, g * GROUP + (c + 1) * CHUNK
                        )
                        nc.tensor.matmul(
                            pslice,
                            lhsT=lhs_sb[:, :, m * 128 : (m + 1) * 128],
                            rhs=rhs_sb[:, :, cols],
                            start=True,
                            stop=False,
                            perf_mode=mybir.MatmulPerfMode.DoubleRow,
                        )
                        bp = 32 * c
                        nc.tensor.matmul(
                            pslice,
                            lhsT=foldl_sb[bp : bp + 4, :, m * 128 : (m + 1) * 128],
                            rhs=foldr_sb[bp : bp + 4, :, cols],
                            start=False,
                            stop=True,
                            perf_mode=mybir.MatmulPerfMode.DoubleRow,
                            tile_position=(bp, 0),
                        )

                    if pipe == "N":
                        t1 = tpool.tile([128, GROUP], _F, tag="t1")
                        nc.vector._custom_dve(
                            op_a,
                            out=t1[:],
                            in0=ps[:],
                            in1=icol_sb[:],
                            s0=a2_col,
                            s1=float(SEED_B),
                            imm2=1.5,
                            accum_out=acc[:, cb : cb + 1],
                        )
                        t2 = tpool.tile([128, GROUP], _F, tag="t1")
                        nc.vector._custom_dve(
                            op_b,
                            out=t2[:],
                            in0=ps[:],
                            in1=b2slice,
                            s0=float(SEED_I),
                            s1=float(SEED_B),
                            imm2=1.5,
                            accum_out=acc[:, cb + 1 : cb + 2],
                        )
                    elif pipe == "B":
                        dt = dpool.tile([128, GROUP], _B, tag="dt")
                        nc.scalar.activation(
                            out=dt[:],
                            in_=ps[:],
                            func=mybir.ActivationFunctionType.Sqrt,
                            scale=2.0,
                        )
                        t1 = tpool.tile([128, GROUP], _F, tag="t1")
                        nc.vector._custom_dve(
                            op_h,
                            out=t1[:],
                            in0=dt[:],
                            in1=bslice,
                            s0=a_col,
                            s1=0.0,
                            imm2=2.0,
                            accum_out=acc[:, cb : cb + 1],
                        )
                    else:  # C
                        dt = dpool.tile([128, GROUP], _B, tag="dt")
                        nc.scalar.activation(
                            out=dt[:],
                            in_=ps[:],
                            func=mybir.ActivationFunctionType.Sqrt,
                            scale=2.0,
                            accum_out=acc[:, cb + 2 : cb + 3],
                        )
                        t1 = tpool.tile([128, GROUP], _B, tag="t1b")
                        nc.gpsimd.scalar_tensor_tensor(
                            out=t1[:],
                            in0=dt[:],
                            scalar=a_col,
                            in1=dt[:],
                            op0=AL.max,
                            op1=AL.max,
                            accum_out=acc[:, cb : cb + 1],
                        )
                        t2 = tpool.tile([128, GROUP], _B, tag="t1b")
                        nc.gpsimd.scalar_tensor_tensor(
                            out=t2[:],
                            in0=dt[:],
                            scalar=-3.0e38,
                            in1=bslice,
                            op0=AL.max,
                            op1=AL.max,
                            accum_out=acc[:, cb + 1 : cb + 2],
                        )

            nc.sync.dma_start(out=out.ap(), in_=acc[:])

    nc.compile()
    return nc


def prepare_inputs(im: np.ndarray, s: np.ndarray):
    """Host-side sharding + dtype conversion. Returns in_maps for 8 cores."""
    im64 = np.ascontiguousarray(im, dtype=np.float64)
    s64 = np.ascontiguousarray(s, dtype=np.float64)

    im_sq = (im64 * im64).sum(1)
    s_sq = (s64 * s64).sum(1)
    diag_true = np.sqrt(((im64 - s64) ** 2).sum(1))
    b_full = MARGIN + diag_true                           # [N] f64 exact

    _f8 = mybir.dt.np(_P8)
    im8 = (-im64).astype(np.float32).astype(_f8)          # negated!
    s8 = s64.astype(np.float32).astype(_f8)

    def resid4(x):
        frs, rem = [], x.copy()
        for _ in range(4):
            r = rem.astype(np.float32).astype(_f8)
            frs.append(r)
            rem = rem - r.astype(np.float64)
        return frs

    fold_s = resid4(0.5 * s_sq)                           # 4 x [N] fp8
    fold_im = resid4(0.5 * im_sq)                         # 4 x [N] fp8

    # foldr [4, 2, N]: slot (p, 0) = s-term p, slot (p, 1) = ones
    foldr = np.zeros((4, 2, N), dtype=_f8)
    for p in range(4):
        foldr[p, 0, :] = fold_s[p]
        foldr[p, 1, :] = np.float32(1.0)

    b_bf = b_full.astype(np.float32).astype(BF16)         # [N] bf16
    b2h_bf = (0.5 * b_full ** 2).astype(np.float32).astype(BF16)
    a_f32 = b_full.astype(np.float32)
    a2h_f32 = (0.5 * b_full ** 2).astype(np.float32)

    sT = np.ascontiguousarray(s8.T.reshape(2, 128, N))    # [i, p, j]

    in_maps = []
    for c in range(NCORES):
        rows = slice(c * SLAB, (c + 1) * SLAB)
        imT = np.ascontiguousarray(im8[rows].T.reshape(2, 128, SLAB))
        foldl = np.zeros((4, 2, SLAB), dtype=_f8)
        for p in range(4):
            foldl[p, 0, :] = np.float32(1.0)
            foldl[p, 1, :] = fold_im[p][rows]
        in_maps.append(
            {
                "imT": imT,
                "sT": sT,
                "foldr": foldr,
                "foldl": foldl,
                "brow": np.ascontiguousarray(b_bf),
                "b2row": np.ascontiguousarray(b2h_bf),
                "avec": np.ascontiguousarray(a_f32[rows].reshape(MT, 128).T),
                "a2vec": np.ascontiguousarray(a2h_f32[rows].reshape(MT, 128).T),
            }
        )
    # host-side constants for the final combine
    consts = {"b_full": b_full, "a_sum_rows": None}
    return in_maps, consts


_NC_CACHE = None


def get_module():
    global _NC_CACHE
    if _NC_CACHE is None:
        _NC_CACHE = build_module()
    return _NC_CACHE


def kernel(im: np.ndarray, s: np.ndarray) -> np.ndarray:
    nc = get_module()
    in_maps, consts = prepare_inputs(im, s)
    b_full = consts["b_full"]
    res = bass_utils.run_bass_kernel_spmd(
        nc, in_maps, core_ids=list(range(NCORES))
    )

    bases, nacc = _acc_layout()
    total = 0.0
    for c in range(NCORES):
        accs = res.results[c]["out"].astype(np.float64)   # [128, nacc]
        col_sums = accs.sum(axis=0)                        # [nacc]
        rows = slice(c * SLAB, (c + 1) * SLAB)
        a_rows = b_full[rows]                              # [SLAB]
        for t, ch in enumerate(PIPE):
            g, m = divmod(t, MT)
            cb = bases[t]
            if ch == "B":
                total += col_sums[cb]
            elif ch == "C":
                total += col_sums[cb] + col_sums[cb + 1] - 2.0 * col_sums[cb + 2]
            else:  # N
                a_m = a_rows[m * 128 : (m + 1) * 128].sum()
                b_g = b_full[g * GROUP : (g + 1) * GROUP].sum()
                total += (
                    GROUP * a_m
                    + 128 * b_g
                    - 2.0 * (1.0 + CA) * col_sums[cb]
                    - 2.0 * (1.0 + CB) * col_sums[cb + 1]
                )
    total -= 2.0 * MARGIN * N
    return np.array(total / (N * N), dtype=np.float32)

